# revision 1
# baseline (speedup 1.0000x reference)
"""Differentiable envelope follower on 8 Trainium2 NeuronCores.

Algorithm: the per-sample recurrence
    env[t] = c[t]*env[t-1] + (1-c[t])*|x[t]|,   c[t] = ca if |x[t]| > env[t-1] else cr
is data-dependent (not a linear scan), but each step is a contraction with
factor <= cr < 1.  We solve it by *policy iteration*: guess the trajectory,
compute the attack/release decisions elementwise from the guess, solve the
resulting LINEAR recurrence exactly with the DVE tensor_tensor_scan
instruction, and repeat.  Decisions stabilize geometrically; 4 iterations converge
geometrically: 3 give ~2e-5 max rel err, 4 reach the ~1.3e-5 floor set by
reference-vs-device rounding (validated at full scale on hardware).

Sharding: L=480000 split across 8 cores (60000 each); within a core, the 64
batch rows x 2 L-halves fill all 128 SBUF partitions ([128, 30000] per core).
Chunk-boundary states are exchanged between iterations with a tiny AllGather
(512B), keeping every iteration's linear solve exact given its decisions.
"""

import math
import numpy as np

# ---- problem constants (hardcoded per contract) ----
B = 64
L = 480000
NCORES = 8
KCORE = L // NCORES          # 60000 per core
HALF = KCORE // 2            # 30000 per partition-half
P = 128

# ---- tunables ----
TF = 1250                    # free-dim tile size (must divide HALF)
BUFS = 3                     # buffers per tile pool
TAILN = 0                    # trailing tiles whose k/d1 run on DVE (0 = off)
ITERS = 4                    # policy iterations
EQ = 1.42                    # equilibrium level of the init guess
TAU = 5600.0                 # time constant of the init-guess ramp
ABS_SPLIT = 0                # tile j does abs on DVE when split>0 and j%split==0, else ACT
DEC_ENGINE = "vector"        # engine for the compare (gpsimd|vector)
D1_POOL = 710                # cols of each d1 tile computed on Pool engine

_RUN_KWARGS = {}             # test.py can set {"trace": True}
_cache = {}


def _coeffs(raw_attack, raw_release, sample_rate):
    # Mirror reference._coefficients exactly (same jax ops, on CPU).
    import jax
    import jax.numpy as jnp

    with jax.default_device(jax.devices("cpu")[0]):
        attack_ms = 0.1 + jax.nn.sigmoid(jnp.asarray(np.float32(raw_attack))) * 49.9
        release_ms = 10.0 + jax.nn.sigmoid(jnp.asarray(np.float32(raw_release))) * 490.0
        attack_samples = attack_ms * float(sample_rate) / 1000.0
        release_samples = release_ms * float(sample_rate) / 1000.0
        ca = jnp.exp(-1.0 / attack_samples)
        cr = jnp.exp(-1.0 / release_samples)
        return float(ca), float(cr)


def _build(ca, cr):
    import concourse.bass as bass
    import concourse.bacc as bacc
    import concourse.tile as tile
    from concourse import mybir

    f32 = mybir.dt.float32
    Alu = mybir.AluOpType
    Act = mybir.ActivationFunctionType
    NT = HALF // TF
    assert NT * TF == HALF

    ka = np.float32(1.0) - np.float32(ca)
    kr = np.float32(1.0) - np.float32(cr)
    dk = float(np.float32(ka) - np.float32(kr))   # k = kr + dk*dec
    dc = float(np.float32(ca) - np.float32(cr))   # c = cr + dc*dec

    nc = bacc.Bacc("TRN2", target_bir_lowering=False, debug=False,
                   num_devices=NCORES)

    x_in = nc.dram_tensor("xc", [P, HALF], f32, kind="ExternalInput")
    seed0_in = nc.dram_tensor("seed0", [P, 1], f32, kind="ExternalInput")
    selw_in = nc.dram_tensor("selw", [P, NCORES], f32, kind="ExternalInput")
    y_out = nc.dram_tensor("yc", [P, HALF], f32, kind="ExternalOutput")
    bnd_loc = nc.dram_tensor("bnd_loc", [P], f32)
    bnd_all = nc.dram_tensor("bnd_all", [NCORES, P], f32, addr_space="Shared")

    dsem = nc.alloc_semaphore("bnd_dma")
    csem = nc.alloc_semaphore("bnd_cc")
    groups = [list(range(NCORES))]

    with tile.TileContext(nc) as tc:
        from contextlib import ExitStack
        with ExitStack() as ctx:
            envp = ctx.enter_context(tc.tile_pool(name="env", bufs=1))
            xp = ctx.enter_context(tc.tile_pool(name="x", bufs=BUFS))
            lp = ctx.enter_context(tc.tile_pool(name="l", bufs=BUFS))
            decp = ctx.enter_context(tc.tile_pool(name="dec", bufs=BUFS))
            kp = ctx.enter_context(tc.tile_pool(name="k", bufs=BUFS))
            cp = ctx.enter_context(tc.tile_pool(name="c", bufs=BUFS))
            d1p = ctx.enter_context(tc.tile_pool(name="d1", bufs=BUFS))
            cst = ctx.enter_context(tc.tile_pool(name="cst", bufs=1))
            bcolp = ctx.enter_context(tc.tile_pool(name="bcol", bufs=2))
            seedp = ctx.enter_context(tc.tile_pool(name="seed", bufs=2))

            env = envp.tile([P, HALF], f32)
            selw_sb = cst.tile([P, NCORES], f32, tag="selw")
            bnd_sb = cst.tile([P, NCORES], f32, tag="bnd")
            sel_t = cst.tile([P, NCORES], f32, tag="sel")

            seed_t = seedp.tile([P, 1], f32, tag="seed")
            nc.gpsimd.dma_start(seed_t[:, :], seed0_in[:, :])
            nc.gpsimd.dma_start(selw_sb[:, :], selw_in[:, :])

            exchanges = 0
            for it in range(ITERS):
                last = it == ITERS - 1
                for j in range(NT):
                    a = j * TF          # global col of first scan output
                    x_t = xp.tile([P, TF], f32, tag="x")
                    nc.sync.dma_start(x_t[:, :], x_in[:, a:a + TF])
                    l_t = lp.tile([P, TF], f32, tag="l")
                    nc.scalar.activation(l_t[:, :], x_t[:, :], Act.Abs)

                    dec_t = decp.tile([P, TF], f32, tag="dec")
                    c_t = cp.tile([P, TF], f32, tag="c")
                    k_t = kp.tile([P, TF], f32, tag="k")
                    d1_t = d1p.tile([P, TF], f32, tag="d1")
                    if it == 0:
                        # vs constant guess EQ: 2x-mode tensor_scalar, no env
                        nc.vector.tensor_scalar(dec_t[:, :], l_t[:, :],
                                                float(EQ), None, op0=Alu.is_gt)
                        nc.scalar.activation(c_t[:, :], dec_t[:, :], Act.Copy,
                                             bias=float(cr), scale=dc)
                        nc.gpsimd.tensor_scalar(k_t[:, :], dec_t[:, :],
                                                dk, float(kr),
                                                op0=Alu.mult, op1=Alu.add)
                        nc.gpsimd.tensor_tensor(d1_t[:, :], k_t[:, :],
                                                l_t[:, :], op=Alu.mult)
                    else:
                        # decisions vs prev-iteration env (col0 vs the newest
                        # value: scan(j-1) output or the exchanged seed)
                        prev_col = env[:, a - 1:a] if j > 0 else seed_t[:, 0:1]
                        nc.vector.tensor_tensor(dec_t[:, 1:], l_t[:, 1:],
                                                env[:, a:a + TF - 1],
                                                op=Alu.is_gt)
                        nc.vector.tensor_tensor(dec_t[:, 0:1], l_t[:, 0:1],
                                                prev_col, op=Alu.is_gt)
                        nc.scalar.activation(c_t[:, 1:], dec_t[:, 1:],
                                             Act.Copy, bias=float(cr), scale=dc)
                        nc.scalar.activation(c_t[:, 0:1], dec_t[:, 0:1],
                                             Act.Copy, bias=float(cr), scale=dc)
                        # tail tiles: Pool lags the iteration; DVE is idle
                        # there, so run their k/d1 on DVE instead
                        kd_eng = nc.vector if j >= NT - TAILN else nc.gpsimd
                        kd_eng.tensor_scalar(k_t[:, 1:], dec_t[:, 1:],
                                             dk, float(kr),
                                             op0=Alu.mult, op1=Alu.add)
                        nc.vector.tensor_scalar(k_t[:, 0:1], dec_t[:, 0:1],
                                                dk, float(kr),
                                                op0=Alu.mult, op1=Alu.add)
                        kd_eng.tensor_tensor(d1_t[:, 1:], k_t[:, 1:],
                                             l_t[:, 1:], op=Alu.mult)
                        nc.vector.tensor_tensor(d1_t[:, 0:1], k_t[:, 0:1],
                                                l_t[:, 0:1], op=Alu.mult)

                    init_ap = env[:, a - 1:a] if j > 0 else seed_t[:, 0:1]
                    nc.vector.tensor_tensor_scan(
                        env[:, a:a + TF], c_t[:, :], d1_t[:, :],
                        init_ap, op0=Alu.mult, op1=Alu.add)

                    if last:
                        nc.sync.dma_start(y_out[:, a:a + TF],
                                          env[:, a:a + TF])

                if not last:
                    # boundary exchange for next iteration's seeds.  No
                    # tile_critical (it drains all engines): manual sems give
                    # timing, add_dep_helper pins Pool-queue order so a
                    # blocked instruction can't starve its own producers.
                    from concourse.tile_rust import add_dep_helper
                    e = exchanges
                    exchanges += 1
                    # snapshot the boundary column so the bnd stores have no
                    # Tile-tracked consumers (their only update is our dsem)
                    bcol = bcolp.tile([P, 1], f32, tag="bcol")
                    nc.vector.tensor_copy(bcol[:, :], env[:, HALF - 1:HALF])
                    st1 = nc.gpsimd.dma_start(
                        bnd_loc[0:64], bcol[64:128, 0:1])
                    st2 = nc.gpsimd.dma_start(
                        bnd_loc[64:128], bcol[0:64, 0:1])
                    cc = nc.gpsimd.collective_compute(
                        "AllGather", mybir.AluOpType.bypass,
                        replica_groups=groups,
                        ins=[bnd_loc[:]], outs=[bnd_all[:, :]],
                    )
                    add_dep_helper(cc.ins, st1.ins, sync=True,
                                   reason="collective after bnd stores")
                    add_dep_helper(cc.ins, st2.ins, sync=True,
                                   reason="collective after bnd stores")
                    for g in range(NCORES):
                        ld = nc.gpsimd.dma_start(bnd_sb[:, g:g + 1],
                                                 bnd_all[g, :])
                        add_dep_helper(ld.ins, cc.ins, sync=True,
                                       reason="bnd load after collective")
                    nc.vector.tensor_tensor(
                        sel_t[:, :], bnd_sb[:, :], selw_sb[:, :],
                        op=mybir.AluOpType.mult)
                    seed_t = seedp.tile([P, 1], f32, tag="seed")
                    nc.vector.tensor_reduce(
                        seed_t[:, :], sel_t[:, :],
                        axis=mybir.AxisListType.X, op=mybir.AluOpType.add)
    nc.finalize()
    return nc


def _in_maps(x, ca, cr):
    x = np.ascontiguousarray(np.asarray(x, dtype=np.float32))
    maps = []
    t0 = np.empty(P, np.float64)
    for c in range(NCORES):
        t0[:64] = c * KCORE
        t0[64:] = c * KCORE + HALF
        seed0 = (EQ * (1.0 - np.exp(-t0 / TAU))).astype(np.float32)[:, None]
        selw = np.zeros((P, NCORES), np.float32)
        if c > 0:
            selw[:64, c - 1] = 1.0
        selw[64:, c] = 1.0
        s = c * KCORE
        xc = np.concatenate([x[:, s:s + HALF], x[:, s + HALF:s + KCORE]], axis=0)
        maps.append({
            "xc": np.ascontiguousarray(xc),
            "seed0": seed0,
            "selw": selw,
        })
    return maps


def kernel(x, raw_attack, raw_release, sample_rate):
    from concourse.bass_utils import run_bass_kernel_spmd

    ca, cr = _coeffs(raw_attack, raw_release, sample_rate)
    key = (round(ca, 12), round(cr, 12), TF, ITERS, ABS_SPLIT, DEC_ENGINE)
    if key not in _cache:
        _cache[key] = _build(ca, cr)
    nc = _cache[key]

    maps = _in_maps(x, ca, cr)
    res = run_bass_kernel_spmd(nc, maps, list(range(NCORES)), **_RUN_KWARGS)
    kernel.last_results = res

    y = np.empty((B, L), np.float32)
    for c in range(NCORES):
        yc = res.results[c]["yc"]
        s = c * KCORE
        y[:, s:s + HALF] = yc[:64]
        y[:, s + HALF:s + KCORE] = yc[64:]
    return y



# revision 9
# speedup vs baseline: 2.3135x; 2.3135x over previous
"""Differentiable envelope follower on 8 Trainium2 NeuronCores.

Algorithm (policy iteration, 3-level):
  env[t] = c[t]*env[t-1] + (1-c[t])*|x[t]|,  c[t] = ca if |x[t]| > env[t-1] else cr.
Each step is a contraction (factor <= cr < 1), so decisions computed from an
approximate trajectory converge geometrically when re-solved with the exact
LINEAR scan (hardware tensor_tensor_scan).

  1. Coarse solve at stride S=15 with a 15000-sample HALO before each chunk
     (host-prepared strided slice) - 2 policy passes, no collectives: seed
     errors decay across the halo.
  2. Full-res pass 1: decisions vs the block-constant coarse env (broadcast
     AP on Pool), scan seeded by the coarse value just before the chunk.
  3. One AllGather exchanges pass-1 chunk boundaries (the only collective),
     overlapped with pre-emitted pass-2 prep.
  4. Full-res pass 2: decisions vs pass-1 env; scan re-seeded; output stored
     as scaled f16, un-scaled on the host.

Numerics: levels and env are stored in f16 SCALED BY 4096 (enables DVE 2x/4x
modes + halves DMA). The scan's internal state is fp32 regardless of operand
dtype, so the recurrence itself does not accumulate f16 error; f16 only
quantizes storage. The x4096 scale keeps tiny env values (t ~ 0) out of the
deep-subnormal range: worst-case rel-err vs the |expected|+1e-9 denominator
is bounded at ~7e-3, inside the 2e-2 gate. Validated in a bit-faithful
prototype: maxrel ~ 5e-3 mid-chunk.

Sharding: L=480000 split 8 ways (60000/core); 64 batch rows x 2 half-chunks
fill the 128 SBUF partitions ([128, 30000] per core).

Engines: scans on Pool (faster than DVE for the scan and dtype-flat),
dec/k/d1/abs on DVE (f16 2x/4x), c on ACT (activation scale+bias),
loads/stores on the SP + ACT HWDGE queues.
"""

import math
import numpy as np

# ---- problem constants ----
B = 64
L = 480000
NCORES = 8
KCORE = L // NCORES          # 60000 per core
HALF = KCORE // 2            # 30000 per partition-row
P = 128

# ---- algorithm constants ----
TF = 1200                    # free-dim tile size (divisible by S)
NT = HALF // TF              # 25 tiles
S = 15                       # coarse stride
HALO = 15000                 # coarse halo (samples) before each chunk
NBH = (HALF + HALO) // S     # 3000 coarse cols per chunk (incl. halo)
NBHALO = HALO // S           # 1000 halo cols
NBT = TF // S                # 80 coarse blocks per full tile
CW = 1500                    # coarse sub-tile width (cols)
NCT = NBH // CW              # coarse sub-tiles
SCALE = 4096.0               # f16 dynamic-range scale
EQ = 1.42                    # equilibrium level of the init guess
TAU = 5600.0                 # ramp time constant of the init guess
CPASSES = 2                  # coarse policy passes
PREP_AHEAD = 2               # pass-2 tiles prepped before the collective
ACT_LOAD_EVERY = 4           # every 4th x-tile load goes on the ACT queue
DEC1_DVE_EVERY = 3           # every 3rd pass-1 dec runs on DVE (Pool relief)

_RUN_KWARGS = {}             # test.py can set {"trace": True}
_cache = {}


def _coeffs(raw_attack, raw_release, sample_rate):
    # Mirror reference._coefficients exactly (same jax ops, on CPU).
    import jax
    import jax.numpy as jnp

    with jax.default_device(jax.devices("cpu")[0]):
        attack_ms = 0.1 + jax.nn.sigmoid(jnp.asarray(np.float32(raw_attack))) * 49.9
        release_ms = 10.0 + jax.nn.sigmoid(jnp.asarray(np.float32(raw_release))) * 490.0
        attack_samples = attack_ms * float(sample_rate) / 1000.0
        release_samples = release_ms * float(sample_rate) / 1000.0
        ca = jnp.exp(-1.0 / attack_samples)
        cr = jnp.exp(-1.0 / release_samples)
        return float(ca), float(cr)


def _build(ca, cr):
    import concourse.bass as bass
    import concourse.bacc as bacc
    import concourse.tile as tile
    from concourse import mybir
    from concourse.tile_rust import add_dep_helper
    from contextlib import ExitStack

    f32 = mybir.dt.float32
    f16 = mybir.dt.float16
    Alu = mybir.AluOpType
    Act = mybir.ActivationFunctionType

    kr = float(np.float32(1.0) - np.float32(cr))
    ka = float(np.float32(1.0) - np.float32(ca))
    dk = float(np.float32(ka) - np.float32(kr))   # k = kr + dk*dec
    dc = float(np.float32(ca) - np.float32(cr))   # c = cr + dc*dec
    caS = float(np.float32(float(np.float64(ca) ** S)))
    crS = float(np.float32(float(np.float64(cr) ** S)))
    krS = float(np.float32(1.0) - np.float32(crS))
    kaS = float(np.float32(1.0) - np.float32(caS))
    dkS = float(np.float32(kaS) - np.float32(krS))
    dcS = float(np.float32(caS) - np.float32(crS))
    eq_s = float(EQ * SCALE)

    nc = bacc.Bacc("TRN2", target_bir_lowering=False, debug=False,
                   num_devices=NCORES)

    x_in = nc.dram_tensor("xc", [P, HALF], f32, kind="ExternalInput")
    xcs_in = nc.dram_tensor("xcs", [P, NBH], f32, kind="ExternalInput")
    cseed_in = nc.dram_tensor("cseed", [P, 1], f32, kind="ExternalInput")
    selw_in = nc.dram_tensor("selw", [P, NCORES], f32, kind="ExternalInput")
    y_out = nc.dram_tensor("yc", [P, HALF], f16, kind="ExternalOutput")
    bnd_loc = nc.dram_tensor("bnd_loc", [P], f32)
    bnd_all = nc.dram_tensor("bnd_all", [NCORES, P], f32, addr_space="Shared")

    with tile.TileContext(nc) as tc:
        with ExitStack() as ctx:
            lsp = ctx.enter_context(tc.tile_pool(name="ls", bufs=1))
            envp = ctx.enter_context(tc.tile_pool(name="env1", bufs=1))
            cstp = ctx.enter_context(tc.tile_pool(name="cst", bufs=1))
            ccp = ctx.enter_context(tc.tile_pool(name="ccrs", bufs=1))
            xp = ctx.enter_context(tc.tile_pool(name="x", bufs=3))
            decp = ctx.enter_context(tc.tile_pool(name="dec", bufs=3))
            kp = ctx.enter_context(tc.tile_pool(name="k", bufs=3))
            cp = ctx.enter_context(tc.tile_pool(name="c", bufs=3))
            d1p = ctx.enter_context(tc.tile_pool(name="d1", bufs=3))
            o2p = ctx.enter_context(tc.tile_pool(name="o2", bufs=2))
            colp = ctx.enter_context(tc.tile_pool(name="col", bufs=4))

            ls = lsp.tile([P, HALF], f16)           # f16(|x| * SCALE)
            env1 = envp.tile([P, HALF], f16)        # pass-1 env (scaled)
            envc = cstp.tile([P, NBH], f16, tag="envc")
            lcs = cstp.tile([P, NBH], f16, tag="lcs")
            cseed = cstp.tile([P, 1], f32, tag="cseed")
            selw_sb = cstp.tile([P, NCORES], f32, tag="selw")
            bnd_sb = cstp.tile([P, NCORES], f32, tag="bnd")
            sel_t = cstp.tile([P, NCORES], f32, tag="sel")

            # ---- coarse data: |xcs| * SCALE as f16 via ACT (sub-tiled) ----
            nc.sync.dma_start(cseed[:, :], cseed_in[:, :])
            nc.sync.dma_start(selw_sb[:, :], selw_in[:, :])
            for t in range(NCT):
                w = t * CW
                xcs_t = ccp.tile([P, CW], f32, tag="xcs")
                nc.sync.dma_start(xcs_t[:, :], xcs_in[:, w:w + CW])
                nc.scalar.activation(lcs[:, w:w + CW], xcs_t[:, :], Act.Abs,
                                     scale=SCALE)

            # ---- coarse policy passes (sub-tiled) ----
            for p in range(CPASSES):
                for t in range(NCT):
                    w = t * CW
                    decc = ccp.tile([P, CW], f16, tag="decc")
                    if p == 0:
                        nc.vector.tensor_scalar(decc[:, :], lcs[:, w:w + CW],
                                                eq_s, None, op0=Alu.is_gt)
                    else:
                        if t == 0:
                            nc.vector.tensor_tensor(
                                decc[:, 1:], lcs[:, w + 1:w + CW],
                                envc[:, w:w + CW - 1], op=Alu.is_gt)
                            nc.vector.tensor_tensor(
                                decc[:, 0:1], lcs[:, w:w + 1],
                                cseed[:, 0:1], op=Alu.is_gt)
                        else:
                            nc.vector.tensor_tensor(
                                decc[:, :], lcs[:, w:w + CW],
                                envc[:, w - 1:w + CW - 1], op=Alu.is_gt)
                    kc = ccp.tile([P, CW], f16, tag="kc")
                    nc.gpsimd.tensor_scalar(kc[:, :], decc[:, :], dkS, krS,
                                            op0=Alu.mult, op1=Alu.add)
                    cc = ccp.tile([P, CW], f32, tag="cc")
                    nc.gpsimd.tensor_scalar(cc[:, :], decc[:, :], dcS, crS,
                                            op0=Alu.mult, op1=Alu.add)
                    nc.gpsimd.tensor_tensor(kc[:, :], kc[:, :],
                                            lcs[:, w:w + CW], op=Alu.mult)
                    init = cseed[:, 0:1] if t == 0 else envc[:, w - 1:w]
                    nc.vector.tensor_tensor_scan(
                        envc[:, w:w + CW], cc[:, :], kc[:, :], init,
                        op0=Alu.mult, op1=Alu.add)

            # ---- pass 1 (prep pipelined 2 tiles ahead of the scan) ----
            def prep1(j):
                a = j * TF
                x_t = xp.tile([P, TF], f32, tag="x")
                eng = nc.scalar if (j % ACT_LOAD_EVERY) == 2 else nc.sync
                eng.dma_start(x_t[:, :], x_in[:, a:a + TF])
                nc.scalar.activation(ls[:, a:a + TF], x_t[:, :], Act.Abs,
                                     scale=SCALE)
                dec_t = decp.tile([P, TF], f16, tag="dec")
                cb0 = NBHALO + j * NBT
                ls3 = ls[:, a:a + TF].rearrange("p (n s) -> p n s", s=S)
                ec3 = envc[:, cb0:cb0 + NBT].rearrange(
                    "p (n s) -> p n s", s=1).broadcast_to([P, NBT, S])
                d3 = dec_t[:, :].rearrange("p (n s) -> p n s", s=S)
                nc.vector.tensor_tensor(d3, ls3, ec3, op=Alu.is_gt)
                k_t = kp.tile([P, TF], f16, tag="k")
                nc.gpsimd.tensor_scalar(k_t[:, :], dec_t[:, :], dk, kr,
                                        op0=Alu.mult, op1=Alu.add)
                c_t = cp.tile([P, TF], f32, tag="c")
                nc.scalar.activation(c_t[:, :], dec_t[:, :], Act.Copy,
                                     bias=cr, scale=dc)
                d1_t = d1p.tile([P, TF], f16, tag="d1")
                nc.gpsimd.tensor_tensor(d1_t[:, :], k_t[:, :],
                                        ls[:, a:a + TF], op=Alu.mult)
                return c_t, d1_t

            q1 = [prep1(0), prep1(1)]
            for j in range(NT):
                a = j * TF
                c_t, d1_t = q1.pop(0)
                if j + 2 < NT:
                    q1.append(prep1(j + 2))
                init = envc[:, NBHALO - 1:NBHALO] if j == 0 else env1[:, a - 1:a]
                nc.vector.tensor_tensor_scan(
                    env1[:, a:a + TF], c_t[:, :], d1_t[:, :], init,
                    op0=Alu.mult, op1=Alu.add)

            # ---- pass 2 prep ----
            def prep2(j):
                a = j * TF
                dec_t = decp.tile([P, TF], f16, tag="dec")
                nc.vector.tensor_tensor(dec_t[:, 1:], ls[:, a + 1:a + TF],
                                        env1[:, a:a + TF - 1], op=Alu.is_gt)
                k_t = kp.tile([P, TF], f16, tag="k")
                nc.gpsimd.tensor_scalar(k_t[:, 1:], dec_t[:, 1:], dk, kr,
                                        op0=Alu.mult, op1=Alu.add)
                c_t = cp.tile([P, TF], f32, tag="c")
                nc.scalar.activation(c_t[:, 1:], dec_t[:, 1:], Act.Copy,
                                     bias=cr, scale=dc)
                d1_t = d1p.tile([P, TF], f16, tag="d1")
                nc.gpsimd.tensor_tensor(d1_t[:, 1:], k_t[:, 1:],
                                        ls[:, a + 1:a + TF], op=Alu.mult)
                return dec_t, k_t, c_t, d1_t

            def fix_col0(j, dec_t, k_t, c_t, d1_t, prevcol):
                a = j * TF
                nc.vector.tensor_tensor(dec_t[:, 0:1], ls[:, a:a + 1],
                                        prevcol, op=Alu.is_gt)
                nc.gpsimd.tensor_scalar(k_t[:, 0:1], dec_t[:, 0:1], dk, kr,
                                        op0=Alu.mult, op1=Alu.add)
                nc.scalar.activation(c_t[:, 0:1], dec_t[:, 0:1], Act.Copy,
                                     bias=cr, scale=dc)
                nc.gpsimd.tensor_tensor(d1_t[:, 0:1], k_t[:, 0:1],
                                        ls[:, a:a + 1], op=Alu.mult)

            pre = {}
            for j in range(PREP_AHEAD):
                pre[j] = prep2(j)
                if j > 0:
                    fix_col0(j, *pre[j], env1[:, j * TF - 1:j * TF])

            # ---- boundary exchange (the single collective) ----
            # bnd_loc[0:64]  = second-half boundary (for the RIGHT core)
            # bnd_loc[64:128] = first-half boundary (for own second half)
            bcol = colp.tile([P, 1], f32, tag="bcol")
            nc.vector.tensor_copy(bcol[:, :], env1[:, HALF - 1:HALF])
            st1 = nc.sync.dma_start(bnd_loc[0:B], bcol[B:P, 0:1])
            st2 = nc.sync.dma_start(bnd_loc[B:P], bcol[0:B, 0:1])
            cc_i = nc.gpsimd.collective_compute(
                "AllGather", mybir.AluOpType.bypass,
                replica_groups=[list(range(NCORES))],
                ins=[bnd_loc[:]], outs=[bnd_all[:, :]],
            )
            add_dep_helper(cc_i.ins, st1.ins, sync=True,
                           reason="collective after bnd stores")
            add_dep_helper(cc_i.ins, st2.ins, sync=True,
                           reason="collective after bnd stores")
            ld = nc.sync.dma_start(bnd_sb[:, :],
                                   bnd_all[:, :].transpose([1, 0]))
            add_dep_helper(ld.ins, cc_i.ins, sync=True,
                           reason="bnd load after collective")
            nc.vector.tensor_tensor(sel_t[:, :], bnd_sb[:, :], selw_sb[:, :],
                                    op=mybir.AluOpType.mult)
            seed2 = colp.tile([P, 1], f32, tag="seed2")
            nc.vector.tensor_reduce(seed2[:, :], sel_t[:, :],
                                    axis=mybir.AxisListType.X,
                                    op=mybir.AluOpType.add)
            fix_col0(0, *pre[0], seed2[:, 0:1])

            # ---- pass 2 scans + stores ----
            prev_out = None
            for j in range(NT):
                a = j * TF
                if j in pre:
                    dec_t, k_t, c_t, d1_t = pre.pop(j)
                else:
                    dec_t, k_t, c_t, d1_t = prep2(j)
                    fix_col0(j, dec_t, k_t, c_t, d1_t, env1[:, a - 1:a])
                o_t = o2p.tile([P, TF], f16, tag="o2")
                init = seed2[:, 0:1] if j == 0 else prev_out[:, TF - 1:TF]
                nc.vector.tensor_tensor_scan(
                    o_t[:, :], c_t[:, :], d1_t[:, :], init,
                    op0=Alu.mult, op1=Alu.add)
                eng = nc.scalar if (j % ACT_LOAD_EVERY) == 1 else nc.sync
                eng.dma_start(y_out[:, a:a + TF], o_t[:, :])
                prev_out = o_t
    nc.finalize()
    return nc


def _in_maps(x):
    x = np.ascontiguousarray(np.asarray(x, dtype=np.float32))
    maps = []
    xe = np.concatenate([np.zeros((B, HALO), np.float32), x], axis=1)
    for c in range(NCORES):
        s = c * KCORE
        xc = np.concatenate([x[:, s:s + HALF], x[:, s + HALF:s + KCORE]],
                            axis=0)
        # coarse strided slices with halo (xe col i <-> global col i-HALO)
        xcs = np.empty((P, NBH), np.float32)
        xcs[:B] = xe[:, s:s + HALO + HALF:S]
        xcs[B:] = xe[:, s + HALF:s + HALF + HALO + HALF:S]
        # coarse seed: ramp value at (chunk_start - HALO), scaled
        cs = np.empty((P, 1), np.float32)
        for half, t0 in ((0, s - HALO), (1, s + HALF - HALO)):
            v = 0.0 if t0 <= 0 else EQ * (1.0 - math.exp(-t0 / TAU))
            cs[half * B:(half + 1) * B] = v * SCALE
        # pass-2 seed selection: rows 0:64 read the LEFT core's chunk-end
        # boundary (bnd_all[c-1][0:64]); rows 64:128 read own first-half
        # boundary (bnd_all[c][64:128]).
        selw = np.zeros((P, NCORES), np.float32)
        if c > 0:
            selw[:B, c - 1] = 1.0
        selw[B:, c] = 1.0
        maps.append({"xc": np.ascontiguousarray(xc),
                     "xcs": np.ascontiguousarray(xcs),
                     "cseed": cs, "selw": selw})
    return maps


def kernel(x, raw_attack, raw_release, sample_rate):
    from concourse.bass_utils import run_bass_kernel_spmd

    ca, cr = _coeffs(raw_attack, raw_release, sample_rate)
    key = (round(ca, 12), round(cr, 12))
    if key not in _cache:
        _cache[key] = _build(ca, cr)
    nc = _cache[key]

    maps = _in_maps(x)
    res = run_bass_kernel_spmd(nc, maps, list(range(NCORES)), **_RUN_KWARGS)
    kernel.last_results = res

    y = np.empty((B, L), np.float32)
    inv = np.float32(1.0 / SCALE)
    for c in range(NCORES):
        yc = res.results[c]["yc"]
        s = c * KCORE
        y[:, s:s + HALF] = yc[:B].astype(np.float32) * inv
        y[:, s + HALF:s + KCORE] = yc[B:].astype(np.float32) * inv
    return y
